# revision 1
# baseline (speedup 1.0000x reference)
"""Trainium2 Bass kernel for MultiLayerCrossModalAttention (v4).

Contract: kernel(**inputs) takes FULL fp32 inputs, returns FULL [B,C,H,W]
fp32 output. Sharding: core = b*2 + half (batch x H-halves); the white/K/V
side carries a 4-pixel halo so attention needs no cross-core traffic.

v4 design (measured-cost driven):
- All convs in bf16 on PE, batched 8x512 into one [C,4096] PSUM tile
  (~0.25us/matmul), drained by ScalarE Identity(+bias) (~8us/4096).
- Software pipelined: layer li+1's K and V convs are both emitted between
  layer li's combine and LayerNorm (A/B-measured best order) so PE/ACT
  conv work hides under DVE streams; Q follows the LN apply it depends on.
- LayerNorm channel stats via ones-matmul on PE (replaces gpsimd
  partition_all_reduce, ~74us/op -> ~10us).
- current_blue eliminated: Q1 = Wq1@blue + (Wq1 diag(g0))@N0 folded into
  one PSUM accumulation group (host-side weight fold).
- attention combine: 9-term accumulation by wide bf16 adds (2x DVE mode)
  instead of 1x-mode reduces; softmax division folded into S per-slice.
- out = blue + acc is finished on HOST in f32 (device acc is bf16 enh sum).
"""

import os
import sys

import numpy as np

if "/opt/trn_rl_repo" not in sys.path:
    sys.path.insert(0, "/opt/trn_rl_repo")

import ml_dtypes

TS = 4
C = 128
NUM_LAYERS = 2
SCALE = float((TS * TS) ** -0.5)
LN_EPS = 1e-5

B, H, W = 4, 128, 128
ROWS = H // 2
KROWS = ROWS + 2 * TS
PW = W + 2 * TS
NTH = ROWS // TS
NTW = W // TS
NTOK = NTH * NTW
NPIX = ROWS * W

_CACHE = {}


def _restride(ap, dim, step):
    b = ap.copy()
    b.ap[dim] = [step, b.ap[dim][1]]
    return b


def _build(reps=1, masked=False):
    import contextlib
    import concourse.bass as bass
    import concourse.tile as tile
    from concourse import bacc, bass_isa, mybir

    if not getattr(bacc, "_act_tables_patched", False):
        _orig_tables = bacc.get_activation_tables
        _KEEP = "natural_log_exp_and_others"

        def _patched(arch):
            t = _orig_tables(arch)
            mine = t[_KEEP]
            return {
                name: (fns if name == _KEEP else (fns - mine))
                for name, fns in t.items()
            }

        bacc.get_activation_tables = _patched
        bacc._act_tables_patched = True

    F32 = mybir.dt.float32
    BF16 = mybir.dt.bfloat16
    AX = mybir.AxisListType
    ALU = mybir.AluOpType
    ACTF = mybir.ActivationFunctionType

    nc = bacc.Bacc("TRN2", target_bir_lowering=False, debug=False, num_devices=8)

    d_blueb = nc.dram_tensor("blueb", [C, NPIX], BF16, kind="ExternalInput").ap()
    d_whiteb = nc.dram_tensor("whiteb", [C, KROWS * W], BF16,
                              kind="ExternalInput").ap()
    d_w = nc.dram_tensor("w", [C, NUM_LAYERS * 4 * C], BF16,
                         kind="ExternalInput").ap()
    d_vecs = nc.dram_tensor("vecs", [C, NUM_LAYERS * 4], F32,
                            kind="ExternalInput").ap()
    d_consts = nc.dram_tensor("consts", [C, 4], F32, kind="ExternalInput").ap()
    d_out = nc.dram_tensor("out", [C, NPIX], BF16, kind="ExternalOutput").ap()

    with tile.TileContext(nc) as tc:
        with (
            nc.allow_low_precision("bf16 compute by design"),
            tc.tile_pool(name="pp", bufs=1) as pp,
            tc.tile_pool(name="psp", bufs=1, space="PSUM") as psp,
        ):
            acc = pp.tile([C, NPIX], BF16)        # 16K: weighted enh sum
            blueb = pp.tile([C, NPIX], BF16)      # 16K
            whiteb = pp.tile([C, KROWS * W], BF16)  # 18K
            Kt = pp.tile([C, KROWS, PW], BF16)    # 19.1K
            Vt = pp.tile([C, KROWS, PW], BF16)    # 19.1K
            T1 = pp.tile([C, NPIX], BF16)         # 16K: Qt
            T2 = pp.tile([C, NPIX], BF16)         # 16K: O / N (normalized)
            S = pp.tile([C, 9, NTOK], BF16)       # 9K
            den = pp.tile([C, NTOK], F32)         # 2K
            axp = pp.tile([C, 3, NTH, NTW, TS], BF16)  # 12K upsampled attn
            big = pp.tile([C, 3, NPIX], BF16)     # 48K: P3 / tmp3 / LN stats
            wts = pp.tile([C, NUM_LAYERS, 4 * C], BF16, name="wts")  # 2K
            vecs = pp.tile([C, NUM_LAYERS, 4], F32, name="vecs")
            consts = pp.tile([C, 4], F32)
            ones = pp.tile([C, C], BF16)
            eps_t = pp.tile([C, 1], F32)

            nc.sync.dma_start(blueb[:], d_blueb[:])
            nc.sync.dma_start(whiteb[:], d_whiteb[:])
            nc.sync.dma_start(wts[:], d_w[:])
            nc.sync.dma_start(vecs[:], d_vecs[:])
            nc.sync.dma_start(consts[:], d_consts[:])
            nc.vector.memset(eps_t[:], LN_EPS)
            nc.vector.memset(ones[:], 1.0)
            mtop = consts[:, 0:1]
            mbot = consts[:, 1:2]
            c0 = consts[:, 2:3]
            # zero x-margins of Kt/Vt once (drains never write them)
            for t in (Kt, Vt):
                m = _restride(
                    t[:, :, 0:TS].unsqueeze(2).broadcast_to(
                        [C, KROWS, 2, TS]), 2, W + TS)
                nc.gpsimd.memset(m, 0.0)

            ps = psp.tile([C, 4096], F32)

            def emit_conv(wmat, bias, dst, src, npx):
                """1x1 conv src->dst via PE + ACT Identity(+bias) drains."""
                px0 = 0
                while px0 < npx:
                    px1 = min(px0 + 4096, npx)
                    for k in range(px0, px1, 512):
                        nc.tensor.matmul(
                            ps[:, k - px0:k - px0 + 512],
                            wmat, src[:, k:k + 512], start=True, stop=True)
                    if dst is None:
                        o = T1[:, px0:px1]
                        i = ps[:, 0:px1 - px0]
                    else:
                        o = dst[:, px0 // W:px1 // W, TS:TS + W]
                        i = ps[:, 0:px1 - px0].rearrange("c (h w) -> c h w", w=W)
                    nc.scalar.activation(o, i, ACTF.Identity, bias=bias)
                    px0 = px1

            def emit_kv_conv(li, which):
                if which == "k":
                    wmat, bias, dst = wts[:, li, C:2 * C], vecs[:, li, 1:2], Kt
                else:
                    wmat, bias, dst = wts[:, li, 2 * C:3 * C], vecs[:, li, 2:3], Vt
                emit_conv(wmat, bias, dst, whiteb[:], KROWS * W)
                if masked:
                    nc.vector.tensor_scalar_mul(
                        dst[:, 0:TS, :], dst[:, 0:TS, :], mtop)
                    nc.vector.tensor_scalar_mul(
                        dst[:, ROWS + TS:KROWS, :],
                        dst[:, ROWS + TS:KROWS, :], mbot)

            def emit_q_conv(li):
                # li0: wq@blue; li1: wq@blue + wqg@N0 (PSUM accumulation)
                wq = wts[:, li, 0:C]
                wqg = wts[:, li, 3 * C:4 * C]
                qb = vecs[:, li, 0:1]
                for px0 in (0, 4096):
                    for k in range(px0, px0 + 4096, 512):
                        nc.tensor.matmul(
                            ps[:, k - px0:k - px0 + 512],
                            wq, blueb[:, k:k + 512],
                            start=True, stop=(li == 0))
                    if li == 1:
                        for k in range(px0, px0 + 4096, 512):
                            nc.tensor.matmul(
                                ps[:, k - px0:k - px0 + 512],
                                wqg, T2[:, k:k + 512],
                                start=False, stop=True,
                                skip_group_check=True)
                    nc.scalar.activation(
                        T1[:, px0:px0 + 4096], ps[:],
                        ACTF.Identity, bias=qb)

            # prologue: layer-0 convs (drain order K, Q, V: logits-di0
            # needs K rows 0:64 + Q; V only needed at combine)
            emit_kv_conv(0, "k")
            emit_q_conv(0)
            emit_kv_conv(0, "v")

            loop = tc.For_i(0, reps, 1) if reps > 1 else contextlib.nullcontext()
            with loop:
                for li in range(NUM_LAYERS):
                    lwg = vecs[:, li, 3:4]

                    # ---- logits: S[3di:3di+3] = blocksum(Q * shift(K))
                    Qv = T1[:].rearrange("c (h w) -> c h w", w=W)
                    for di in range(3):
                        qb3 = Qv.unsqueeze(1).broadcast_to([C, 3, ROWS, W])
                        kb3 = _restride(
                            Kt[:, 4 * di:4 * di + ROWS, 0:W]
                            .unsqueeze(1).broadcast_to([C, 3, ROWS, W]),
                            1, TS)
                        p3v = big[:].rearrange("c n (h w) -> c n h w", w=W)
                        nc.vector.tensor_mul(p3v, qb3, kb3)
                        v = p3v.rearrange(
                            "c dj (th r) (tw s) -> c dj th tw r s",
                            r=TS, s=TS).rearrange(
                            "c dj th tw r s -> c (dj th) tw r s")
                        nc.vector.reduce_sum(
                            S[:, 3 * di:3 * di + 3, :], v, axis=AX.XY)

                    # ---- softmax over 9 neighbors (logits O(0.3), no max)
                    sf = S[:].rearrange("c n t -> c (n t)")
                    nc.scalar.activation(sf, sf, ACTF.Exp, scale=SCALE)
                    nc.vector.reduce_sum(
                        den[:], S[:].rearrange("c n t -> c t n"), axis=AX.X)
                    nc.vector.reciprocal(den[:], den[:])
                    # divide S by den per-slice so upsample di=0 starts
                    # before the full S is scaled
                    db = den[:].unsqueeze(1)
                    nc.vector.tensor_mul(
                        S[:, 0:3], S[:, 0:3], db.broadcast_to([C, 3, NTOK]))
                    nc.vector.tensor_mul(
                        S[:, 3:9], S[:, 3:9], db.broadcast_to([C, 6, NTOK]))

                    # ---- combine: O = sum_n upsample(A_n) * shift_n(V)
                    O = T2[:]
                    for di in range(3):
                        src = S[:, 3 * di:3 * di + 3, :].rearrange(
                            "c n (th tw) -> c n th tw", tw=NTW).unsqueeze(
                            4).broadcast_to([C, 3, NTH, NTW, TS])
                        nc.vector.tensor_copy(axp[:], src)
                        t3v = big[:].rearrange(
                            "c n (th r x) -> c n th r x", r=TS, x=W)
                        vap = _restride(
                            Vt[:, 4 * di:4 * di + ROWS, 0:W]
                            .unsqueeze(1).broadcast_to([C, 3, ROWS, W]),
                            1, TS).rearrange(
                            "c n (th r) x -> c n th r x", r=TS)
                        aap = axp[:].rearrange(
                            "c n th tw s -> c n th (tw s)").unsqueeze(
                            3).broadcast_to([C, 3, NTH, TS, W])
                        nc.vector.tensor_mul(t3v, vap, aap)
                        if di == 0:
                            nc.vector.tensor_add(O, big[:, 0, :], big[:, 1, :])
                            nc.vector.tensor_add(O, O, big[:, 2, :])
                        else:
                            for n in range(3):
                                nc.vector.tensor_add(O, O, big[:, n, :])

                    # ---- next layer's K conv: PE+ACT work that hides
                    # under this layer's LN DVE stream
                    nli = 1 - li
                    emit_next = (li == 0) or reps > 1
                    if emit_next:
                        emit_kv_conv(nli, "k")
                        emit_kv_conv(nli, "v")

                    # ---- LayerNorm over C via ones-matmul stats
                    o2 = big[:, 0, :]
                    mun = big[:, 1, :]
                    istd = big[:, 2, :]
                    nc.scalar.activation(o2, T2[:], ACTF.Square)
                    for (srcT, dstv, scl) in ((T2[:], mun, -1.0 / C),
                                              (o2, istd, 1.0 / C)):
                        for hx in (0, 4096):
                            for k in range(hx, hx + 4096, 512):
                                nc.tensor.matmul(
                                    ps[:, k - hx:k - hx + 512],
                                    ones[:], srcT[:, k:k + 512],
                                    start=True, stop=True)
                            nc.scalar.activation(
                                dstv[:, hx:hx + 4096], ps[:],
                                ACTF.Identity, scale=scl)
                    # istd holds E[x^2]; mun holds -mu
                    nc.scalar.activation(o2, mun, ACTF.Square)  # mu^2
                    nc.vector.tensor_add(T2[:], T2[:], mun)     # O - mu
                    nc.vector.tensor_sub(istd, istd, o2)        # var
                    nc.scalar.activation(istd, istd, ACTF.Ln, bias=eps_t[:])
                    nc.scalar.activation(istd, istd, ACTF.Exp, scale=-0.5)
                    nc.vector.tensor_mul(T2[:], T2[:], istd)    # N
                    if li == 0:
                        nc.vector.tensor_scalar(
                            acc[:], T2[:], lwg, c0, op0=ALU.mult, op1=ALU.add)
                    else:
                        nc.vector.scalar_tensor_tensor(
                            acc[:], T2[:], lwg, acc[:],
                            op0=ALU.mult, op1=ALU.add)
                    if emit_next:
                        emit_q_conv(nli)

            nc.sync.dma_start(d_out[:], acc[:])

    nc.compile()
    return nc


def _prep_inputs(blue, white, q_w, q_b, k_w, k_b, v_w, v_b, ln_g, ln_b,
                 layer_weights):
    bf16 = ml_dtypes.bfloat16
    f32 = np.float32

    blue = np.asarray(blue, f32)
    whiteP = np.zeros((B, C, H + 2 * TS, W), dtype=f32)
    whiteP[:, :, TS:TS + H, :] = np.asarray(white, f32)

    q_w = np.asarray(q_w, f32)
    q_b = np.asarray(q_b, f32)
    k_w = np.asarray(k_w, f32)
    v_w = np.asarray(v_w, f32)
    ln_b = np.asarray(ln_b, f32)
    ln_g = np.asarray(ln_g, f32)
    lwv = np.asarray(layer_weights, f32)

    wpack = np.zeros((C, NUM_LAYERS, 4 * C), dtype=bf16)
    for li in range(NUM_LAYERS):
        wpack[:, li, 0:C] = q_w[li].T.astype(bf16)
        wpack[:, li, C:2 * C] = k_w[li].T.astype(bf16)
        wpack[:, li, 2 * C:3 * C] = v_w[li].T.astype(bf16)
    # Q1 = Wq1@blue + (Wq1 diag(g0))@N0  (+ qb1 + Wq1@b0)
    wpack[:, 1, 3 * C:4 * C] = (q_w[1].T * ln_g[0][:, None]).astype(bf16)

    vecs = np.zeros((C, NUM_LAYERS, 4), dtype=f32)
    vecs[:, 0, 0] = q_b[0]
    vecs[:, 1, 0] = q_b[1] + q_w[1] @ ln_b[0]
    vecs[:, :, 1] = np.asarray(k_b, f32).T
    vecs[:, :, 2] = np.asarray(v_b, f32).T
    vecs[:, :, 3] = (ln_g * lwv.reshape(NUM_LAYERS, 1)).T

    in_maps = []
    for core in range(8):
        b, half = core // 2, core % 2
        y0 = half * ROWS
        consts = np.zeros((C, 4), f32)
        consts[:, 0] = 0.0 if half == 0 else 1.0
        consts[:, 1] = 0.0 if half == 1 else 1.0
        consts[:, 2] = ln_b[0] * lwv[0] + ln_b[1] * lwv[1]
        in_maps.append({
            "blueb": np.ascontiguousarray(
                blue[b, :, y0:y0 + ROWS, :]).reshape(C, NPIX).astype(bf16),
            "whiteb": np.ascontiguousarray(
                whiteP[b, :, y0:y0 + KROWS, :]).reshape(
                C, KROWS * W).astype(bf16),
            "w": wpack.reshape(C, NUM_LAYERS * 4 * C),
            "vecs": vecs.reshape(C, NUM_LAYERS * 4),
            "consts": consts,
        })
    return in_maps


def kernel(**inputs):
    from concourse.bass_utils import run_bass_kernel_spmd

    reps = int(os.environ.get("KBENCH_REPS", "1"))
    masked = bool(
        np.any(np.asarray(inputs["k_b"])) or np.any(np.asarray(inputs["v_b"])))
    key = ("nc", reps, masked)
    if key not in _CACHE:
        _CACHE[key] = _build(reps, masked)
    nc = _CACHE[key]

    in_maps = _prep_inputs(**inputs)
    res = run_bass_kernel_spmd(nc, in_maps, core_ids=list(range(8)))

    blue = np.asarray(inputs["blue"], np.float32)
    out = np.empty((B, C, H, W), np.float32)
    for core in range(8):
        b, half = core // 2, core % 2
        y0 = half * ROWS
        enh = np.asarray(res.results[core]["out"],
                         np.float32).reshape(C, ROWS, W)
        out[b, :, y0:y0 + ROWS, :] = blue[b, :, y0:y0 + ROWS, :] + enh
    return out



# revision 11
# speedup vs baseline: 1.1780x; 1.1780x over previous
"""Trainium2 Bass kernel for MultiLayerCrossModalAttention (v5).

Contract: kernel(**inputs) takes FULL fp32 inputs, returns FULL [B,C,H,W]
fp32 output. Sharding: core = b*2 + half (batch x H-halves); the white/K/V
side carries a 4-pixel halo so attention needs no cross-core traffic.

v5 design (cost-model driven; v4 was DVE-bound at ~455us busy):
- Logits: Q*K products written TOKEN-GROUPED ([th,tw,(r,s)] layout, still
  2x DVE mode) + contiguous pairwise-halves reduction tree at 2x mode,
  replacing 1x-mode reduce_sum (25.7us -> ~13us per di-group). Tree levels
  L3/L4 run on the otherwise-idle GPSIMD engine.
- Softmax: A = E/den computed at TOKEN resolution (one small 2x mul);
  upsample to pixel res via ACT strided copies (ACT is AP-driven, 1x
  regardless). No pixel-res dinv multiply at all.
- Combine: 9 A*V products per 2048-px strip on DVE (2x), accumulated in
  PSUM by identity-weight matmuls on the idle PE (replaces 8 DVE adds
  per layer, ~35us/layer).
- LayerNorm: centering via M = I - J/C matmul on PE; var = E[Xc^2] via
  ones/C matmul (no mu^2 cancellation); istd' = Exp(-0.5*Ln(var+eps) +
  ln(g*lw)) with the affine gain folded into the ACT bias host-side.
  PSUM drains for center/var go to GPSIMD; chunked 4x for pipelining.
- current_blue eliminated: Q1 = Wq1@blue + (Wq1/lw0)@Ng0 folded into one
  PSUM accumulation group (Ng0 = lw0*g0*N0 is the layer-0 output tensor).
- out = blue + acc + sum(lw_i*b_i) finished on HOST in f32.
"""

import os
import sys

import numpy as np

if "/opt/trn_rl_repo" not in sys.path:
    sys.path.insert(0, "/opt/trn_rl_repo")

import ml_dtypes

TS = 4
C = 128
NUM_LAYERS = 2
SCALE = float((TS * TS) ** -0.5)
LN_EPS = 1e-5

B, H, W = 4, 128, 128
ROWS = H // 2
KROWS = ROWS + 2 * TS
PW = W + 2 * TS
NTH = ROWS // TS
NTW = W // TS
NTOK = NTH * NTW
NPIX = ROWS * W

_CACHE = {}


def _restride(ap, dim, step):
    b = ap.copy()
    b.ap[dim] = [step, b.ap[dim][1]]
    return b


def _build(reps=1, masked=False, possign=True):
    import contextlib
    import concourse.bass as bass  # noqa: F401
    import concourse.tile as tile
    from concourse import bacc, mybir

    if not getattr(bacc, "_act_tables_patched", False):
        _orig_tables = bacc.get_activation_tables
        _KEEP = "natural_log_exp_and_others"

        def _patched(arch):
            t = _orig_tables(arch)
            mine = t[_KEEP]
            return {
                name: (fns if name == _KEEP else (fns - mine))
                for name, fns in t.items()
            }

        bacc.get_activation_tables = _patched
        bacc._act_tables_patched = True

    F32 = mybir.dt.float32
    BF16 = mybir.dt.bfloat16
    ACTF = mybir.ActivationFunctionType

    nc = bacc.Bacc("TRN2", target_bir_lowering=False, debug=False,
                   num_devices=8)

    d_blueb = nc.dram_tensor("blueb", [C, NPIX], BF16,
                             kind="ExternalInput").ap()
    d_whiteb = nc.dram_tensor("whiteb", [C, KROWS * W], BF16,
                              kind="ExternalInput").ap()
    d_w = nc.dram_tensor("w", [C, 1280], BF16, kind="ExternalInput").ap()
    d_vecs = nc.dram_tensor("vecs", [C, 12], F32, kind="ExternalInput").ap()
    d_consts = nc.dram_tensor("consts", [C, 2], F32,
                              kind="ExternalInput").ap()
    d_out = nc.dram_tensor("out", [C, NPIX], BF16, kind="ExternalOutput").ap()

    with tile.TileContext(nc) as tc:
        with (
            nc.allow_low_precision("bf16 compute by design"),
            tc.tile_pool(name="pp", bufs=1) as pp,
            tc.tile_pool(name="psp", bufs=1, space="PSUM") as psp,
        ):
            blueb = pp.tile([C, NPIX], BF16)            # 16K
            whiteb = pp.tile([C, KROWS * W], BF16)      # 18K
            wts = pp.tile([C, 1280], BF16)              # 2.5K
            vecs = pp.tile([C, 12], F32)
            consts = pp.tile([C, 2], F32)
            eps_t = pp.tile([C, 1], F32)
            Kbuf = pp.tile([C, KROWS, PW], BF16)        # 19.1K
            Vbuf = pp.tile([C, KROWS, PW], BF16)        # 19.1K
            T1q = pp.tile([C, NPIX], BF16)              # 16K  Q / O / Xc
            Ng0 = pp.tile([C, NPIX], BF16)              # 16K  layer-0 out
            Ph = pp.tile([C, 32, 128], BF16)            # 8K   QK products
            T1h = pp.tile([C, 32, 32, 2], BF16)         # 4K   s-pair sums
            T2h = [pp.tile([C, 32, 32], BF16, name=f"t2h{i}")
                   for i in range(2)]                   # 2x2K s sums
            U1h = pp.tile([C, 8, 2, 32], BF16)          # 1K   r-pair sums
            S = pp.tile([C, 9, NTOK], BF16)             # 9K logits/E/A
            axp = pp.tile([C, 9, NTH, NTW, TS], BF16)   # 36K upsampled A
            dL2 = pp.tile([C, 2, NTOK], BF16)           # 2K
            den = pp.tile([C, NTOK], BF16)              # 1K
            dinv = pp.tile([C, NTOK], BF16)             # 1K
            cb = [pp.tile([C, 2048], BF16, name=f"cb{i}") for i in range(3)]
            varb = pp.tile([C, NPIX], BF16)             # 16K var/istd

            ps_a = psp.tile([C, 2048], F32)
            ps_b = psp.tile([C, 2048], F32)

            nc.sync.dma_start(blueb[:], d_blueb[:])
            nc.sync.dma_start(whiteb[:], d_whiteb[:])
            nc.sync.dma_start(wts[:], d_w[:])
            nc.sync.dma_start(vecs[:], d_vecs[:])
            nc.sync.dma_start(consts[:], d_consts[:])
            nc.vector.memset(eps_t[:], LN_EPS)
            mtop = consts[:, 0:1]
            mbot = consts[:, 1:2]
            # zero x-margins of K/V once (drains never write them)
            for t in (Kbuf, Vbuf):
                m = _restride(
                    t[:, :, 0:TS].unsqueeze(2).broadcast_to(
                        [C, KROWS, 2, TS]), 2, W + TS)
                nc.gpsimd.memset(m, 0.0)

            # weight slices
            def wmat(i):
                return wts[:, 128 * i:128 * (i + 1)]

            WQ = [wmat(0), wmat(3)]
            WK = [wmat(1), wmat(4)]
            WV = [wmat(2), wmat(5)]
            WQG = wmat(6)
            MC = wmat(7)      # I - J/C
            ONESC = wmat(8)   # 1/C
            IDENT = wmat(9)

            qb = [vecs[:, 0:1], vecs[:, 1:2]]
            kb = [vecs[:, 2:3], vecs[:, 3:4]]
            vb = [vecs[:, 4:5], vecs[:, 5:6]]
            lnb = [vecs[:, 6:7], vecs[:, 7:8]]
            sv = [vecs[:, 8:9], vecs[:, 9:10]]

            def emit_kv_conv(li, which):
                w = WK[li] if which == "k" else WV[li]
                bias = kb[li] if which == "k" else vb[li]
                dst = Kbuf if which == "k" else Vbuf
                npx = KROWS * W
                px0 = 0
                while px0 < npx:
                    px1 = min(px0 + 2048, npx)
                    for k in range(px0, px1, 512):
                        nc.tensor.matmul(
                            ps_a[:, k - px0:k - px0 + 512],
                            w, whiteb[:, k:k + 512], start=True, stop=True)
                    o = dst[:, px0 // W:px1 // W, TS:TS + W]
                    i = ps_a[:, 0:px1 - px0].rearrange(
                        "c (h w) -> c h w", w=W)
                    nc.scalar.activation(o, i, ACTF.Identity, bias=bias)
                    px0 = px1
                if masked:
                    nc.vector.tensor_scalar_mul(
                        dst[:, 0:TS, :], dst[:, 0:TS, :], mtop)
                    nc.vector.tensor_scalar_mul(
                        dst[:, ROWS + TS:KROWS, :],
                        dst[:, ROWS + TS:KROWS, :], mbot)

            def emit_q_conv(li):
                for px0 in range(0, NPIX, 2048):
                    for k in range(px0, px0 + 2048, 512):
                        nc.tensor.matmul(
                            ps_a[:, k - px0:k - px0 + 512],
                            WQ[li], blueb[:, k:k + 512],
                            start=True, stop=(li == 0))
                    if li == 1:
                        for k in range(px0, px0 + 2048, 512):
                            nc.tensor.matmul(
                                ps_a[:, k - px0:k - px0 + 512],
                                WQG, Ng0[:, k:k + 512],
                                start=False, stop=True,
                                skip_group_check=True)
                    nc.scalar.activation(
                        T1q[:, px0:px0 + 2048], ps_a[:],
                        ACTF.Identity, bias=qb[li])

            # prologue: layer-0 convs
            emit_kv_conv(0, "k")
            emit_q_conv(0)
            emit_kv_conv(0, "v")

            loop = tc.For_i(0, reps, 1) if reps > 1 else contextlib.nullcontext()
            with loop:
                for li in range(NUM_LAYERS):
                    nli = 1 - li
                    emit_next = (li == 0) or reps > 1

                    # ---- logits: S[c, 3di+dj, t] = blocksum(Q * shift(K))
                    # pixel-layout products + pairwise tree (2x where legal)
                    Qpix = T1q[:].rearrange("c (y x) -> c y x", x=W)

                    def emit_rtree(di, hh, dj):
                        t2v = T2h[dj % 2][:].rearrange(
                            "c (th r) tw -> c th r tw", r=4)
                        eng = nc.gpsimd if dj == 1 else nc.vector
                        eng.tensor_add(
                            U1h[:], t2v[:, :, 0:2, :], t2v[:, :, 2:4, :])
                        so = S[:, 3 * di + dj,
                               256 * hh:256 * hh + 256].rearrange(
                            "c (th tw) -> c th tw", tw=32)
                        nc.vector.tensor_add(
                            so, U1h[:, :, 0, :], U1h[:, :, 1, :])

                    for di in range(3):
                        for hh in range(2):
                            y0 = 32 * hh
                            for dj in range(3):
                                kv = Kbuf[:, 4 * di + y0:4 * di + y0 + 32,
                                          4 * dj:4 * dj + W]
                                nc.vector.tensor_mul(
                                    Ph[:], Qpix[:, y0:y0 + 32, :], kv)
                                P4 = Ph[:].rearrange(
                                    "c y (tw s) -> c y tw s", s=4)
                                nc.vector.tensor_add(
                                    T1h[:], P4[:, :, :, 0:2], P4[:, :, :, 2:4])
                                t2 = T2h[dj % 2]
                                nc.gpsimd.tensor_add(
                                    t2[:], T1h[:, :, :, 0], T1h[:, :, :, 1])
                                if dj >= 1:
                                    emit_rtree(di, hh, dj - 1)
                            emit_rtree(di, hh, 2)

                    # ---- softmax at token res: A = exp(s*S) / den
                    nc.scalar.activation(
                        S[:].rearrange("c n t -> c (n t)"),
                        S[:].rearrange("c n t -> c (n t)"),
                        ACTF.Exp, scale=SCALE)
                    dL1 = Ph[:].rearrange(
                        "c a b -> c (a b)")[:, 0:2048].rearrange(
                        "c (n t) -> c n t", t=NTOK)
                    nc.vector.tensor_add(dL1, S[:, 0:4], S[:, 4:8])
                    nc.vector.tensor_add(dL2[:], dL1[:, 0:2], dL1[:, 2:4])
                    nc.vector.tensor_add(den[:], dL2[:, 0], dL2[:, 1])
                    nc.vector.tensor_add(den[:], den[:], S[:, 8])
                    nc.vector.reciprocal(dinv[:], den[:])
                    nc.vector.tensor_mul(
                        S[:], S[:],
                        dinv[:].unsqueeze(1).broadcast_to([C, 9, NTOK]))
                    # upsample A -> axp via ACT strided copies, per th-band
                    for st in range(4):
                        for s_ in range(4):
                            nc.scalar.copy(
                                axp[:, :, 4 * st:4 * st + 4, :, s_],
                                S[:, :, 128 * st:128 * st + 128].rearrange(
                                    "c n (th tw) -> c n th tw", tw=32))

                    # next layer's K conv hides under combine DVE stream
                    if emit_next:
                        emit_kv_conv(nli, "k")

                    # ---- combine: O = sum_n upsample(A_n) * shift_n(V)
                    # DVE products per strip; PE identity-matmul PSUM acc
                    for st in range(4):
                        for n in range(9):
                            di, dj = n // 3, n % 3
                            b = cb[n % 3]
                            vsl = Vbuf[:, 16 * st + 4 * di:
                                       16 * st + 4 * di + 16,
                                       4 * dj:4 * dj + 128].rearrange(
                                "c (th r) x -> c th r x", r=4)
                            asl = axp[:, n, 4 * st:4 * st + 4, :, :].rearrange(
                                "c th tw s -> c th (tw s)").unsqueeze(
                                2).broadcast_to([C, 4, 4, 128])
                            nc.vector.tensor_mul(
                                b[:].rearrange(
                                    "c (th r x) -> c th r x", r=4, x=128),
                                asl, vsl)
                            for c4 in range(4):
                                nc.tensor.matmul(
                                    ps_b[:, 512 * c4:512 * c4 + 512],
                                    IDENT, b[:, 512 * c4:512 * c4 + 512],
                                    start=(n == 0), stop=(n == 8),
                                    skip_group_check=(n > 0))
                        nc.scalar.activation(
                            T1q[:, 2048 * st:2048 * st + 2048], ps_b[:],
                            ACTF.Copy)

                    if emit_next:
                        emit_kv_conv(nli, "v")

                    # ---- LayerNorm tail, chunk-pipelined
                    for ch in range(4):
                        c0, c1 = 2048 * ch, 2048 * ch + 2048
                        for k in range(c0, c1, 512):
                            nc.tensor.matmul(
                                ps_a[:, k - c0:k - c0 + 512],
                                MC, T1q[:, k:k + 512], start=True, stop=True)
                        nc.vector.tensor_scalar_mul(
                            T1q[:, c0:c1], ps_a[:], 1.0)  # Xc
                        nc.vector.tensor_mul(
                            cb[2][:], T1q[:, c0:c1], T1q[:, c0:c1])  # Xc^2
                        for c4 in range(4):
                            nc.tensor.matmul(
                                ps_b[:, 512 * c4:512 * c4 + 512],
                                ONESC, cb[2][:, 512 * c4:512 * c4 + 512],
                                start=True, stop=True)
                        nc.scalar.activation(
                            varb[:, c0:c1], ps_b[:], ACTF.Copy)
                        nc.scalar.activation(
                            varb[:, c0:c1], varb[:, c0:c1],
                            ACTF.Ln, bias=eps_t[:])
                        nc.scalar.activation(
                            varb[:, c0:c1], varb[:, c0:c1],
                            ACTF.Exp, scale=-0.5, bias=lnb[li])
                        if li == 0:
                            nc.vector.tensor_mul(
                                Ng0[:, c0:c1], T1q[:, c0:c1], varb[:, c0:c1])
                        else:
                            nc.vector.tensor_mul(
                                T1q[:, c0:c1], T1q[:, c0:c1], varb[:, c0:c1])

                    if li == 0:
                        if emit_next:
                            emit_q_conv(1)
                    else:
                        if not possign:
                            nc.vector.tensor_scalar_mul(
                                Ng0[:], Ng0[:], sv[0])
                            nc.vector.tensor_scalar_mul(
                                T1q[:], T1q[:], sv[1])
                        nc.vector.tensor_add(Ng0[:], Ng0[:], T1q[:])
                        if reps > 1:
                            emit_kv_conv(0, "k")
                            emit_q_conv(0)
                            emit_kv_conv(0, "v")

                nc.sync.dma_start(d_out[:], Ng0[:])

    nc.compile()
    return nc


def _prep_inputs(blue, white, q_w, q_b, k_w, k_b, v_w, v_b, ln_g, ln_b,
                 layer_weights):
    bf16 = ml_dtypes.bfloat16
    f32 = np.float32

    blue = np.asarray(blue, f32)
    whiteP = np.zeros((B, C, H + 2 * TS, W), dtype=f32)
    whiteP[:, :, TS:TS + H, :] = np.asarray(white, f32)

    q_w = np.asarray(q_w, f32)
    q_b = np.asarray(q_b, f32)
    k_w = np.asarray(k_w, f32)
    k_b = np.asarray(k_b, f32)
    v_w = np.asarray(v_w, f32)
    v_b = np.asarray(v_b, f32)
    ln_b = np.asarray(ln_b, f32)
    ln_g = np.asarray(ln_g, f32)
    lwv = np.asarray(layer_weights, f32)

    possign = bool(lwv[0] > 0 and lwv[1] > 0 and np.all(ln_g > 0))

    wpack = np.zeros((C, 1280), dtype=bf16)
    wpack[:, 0:128] = q_w[0].T.astype(bf16)
    wpack[:, 128:256] = k_w[0].T.astype(bf16)
    wpack[:, 256:384] = v_w[0].T.astype(bf16)
    wpack[:, 384:512] = q_w[1].T.astype(bf16)
    wpack[:, 512:640] = k_w[1].T.astype(bf16)
    wpack[:, 640:768] = v_w[1].T.astype(bf16)
    if possign:
        # Ng0 = lw0*g0*N0 ; Q1 needs g0*N0 = Ng0/lw0
        wpack[:, 768:896] = (q_w[1].T / lwv[0]).astype(bf16)
    else:
        # Ng0 = |g0|*N0 ; Q1 needs g0*N0 = sign(g0)*Ng0
        wpack[:, 768:896] = (q_w[1].T
                             * np.sign(ln_g[0])[:, None]).astype(bf16)
    wpack[:, 896:1024] = (np.eye(C, dtype=f32) - 1.0 / C).astype(bf16)
    wpack[:, 1024:1152] = np.full((C, C), 1.0 / C, dtype=f32).astype(bf16)
    wpack[:, 1152:1280] = np.eye(C, dtype=f32).astype(bf16)

    vecs = np.zeros((C, 12), dtype=f32)
    vecs[:, 0] = q_b[0]
    vecs[:, 1] = q_b[1] + q_w[1] @ ln_b[0]
    vecs[:, 2] = k_b[0]
    vecs[:, 3] = k_b[1]
    vecs[:, 4] = v_b[0]
    vecs[:, 5] = v_b[1]
    with np.errstate(divide="ignore"):
        if possign:
            vecs[:, 6] = np.log(lwv[0] * ln_g[0])
            vecs[:, 7] = np.log(lwv[1] * ln_g[1])
        else:
            vecs[:, 6] = np.log(np.abs(ln_g[0]))
            vecs[:, 7] = np.log(np.abs(ln_g[1]))
    vecs[:, 8] = lwv[0] * np.sign(ln_g[0])
    vecs[:, 9] = lwv[1] * np.sign(ln_g[1])

    in_maps = []
    for core in range(8):
        b, half = core // 2, core % 2
        y0 = half * ROWS
        consts = np.zeros((C, 2), f32)
        consts[:, 0] = 0.0 if half == 0 else 1.0
        consts[:, 1] = 0.0 if half == 1 else 1.0
        in_maps.append({
            "blueb": np.ascontiguousarray(
                blue[b, :, y0:y0 + ROWS, :]).reshape(C, NPIX).astype(bf16),
            "whiteb": np.ascontiguousarray(
                whiteP[b, :, y0:y0 + KROWS, :]).reshape(
                C, KROWS * W).astype(bf16),
            "w": wpack,
            "vecs": vecs,
            "consts": consts,
        })
    return in_maps, possign


def kernel(**inputs):
    from concourse.bass_utils import run_bass_kernel_spmd

    reps = int(os.environ.get("KBENCH_REPS", "1"))
    masked = bool(
        np.any(np.asarray(inputs["k_b"])) or np.any(np.asarray(inputs["v_b"])))
    in_maps, possign = _prep_inputs(**inputs)
    key = ("nc", reps, masked, possign)
    if key not in _CACHE:
        _CACHE[key] = _build(reps, masked, possign)
    nc = _CACHE[key]

    res = run_bass_kernel_spmd(nc, in_maps, core_ids=list(range(8)))

    blue = np.asarray(inputs["blue"], np.float32)
    ln_b = np.asarray(inputs["ln_b"], np.float32)
    lwv = np.asarray(inputs["layer_weights"], np.float32)
    c0 = (lwv[0] * ln_b[0] + lwv[1] * ln_b[1]).astype(np.float32)
    out = np.empty((B, C, H, W), np.float32)
    for core in range(8):
        b, half = core // 2, core % 2
        y0 = half * ROWS
        enh = np.asarray(res.results[core]["out"],
                         np.float32).reshape(C, ROWS, W)
        out[b, :, y0:y0 + ROWS, :] = (
            blue[b, :, y0:y0 + ROWS, :] + enh + c0[:, None, None])
    return out


# revision 22
# speedup vs baseline: 1.3881x; 1.1784x over previous
"""Trainium2 Bass kernel for MultiLayerCrossModalAttention (v5).

Contract: kernel(**inputs) takes FULL fp32 inputs, returns FULL [B,C,H,W]
fp32 output. Sharding: core = b*2 + half (batch x H-halves); the white/K/V
side carries a 4-pixel halo so attention needs no cross-core traffic.

v5 design (cost-model driven; v4 was DVE-bound at ~455us busy):
- Logits: Q*K products written TOKEN-GROUPED ([th,tw,(r,s)] layout, still
  2x DVE mode) + contiguous pairwise-halves reduction tree at 2x mode,
  replacing 1x-mode reduce_sum (25.7us -> ~13us per di-group). Tree levels
  L3/L4 run on the otherwise-idle GPSIMD engine.
- Softmax: A = E/den computed at TOKEN resolution (one small 2x mul);
  upsample to pixel res via ACT strided copies (ACT is AP-driven, 1x
  regardless). No pixel-res dinv multiply at all.
- Combine: 9 A*V products per 2048-px strip on DVE (2x), accumulated in
  PSUM by identity-weight matmuls on the idle PE (replaces 8 DVE adds
  per layer, ~35us/layer).
- LayerNorm: centering via M = I - J/C matmul on PE; var = E[Xc^2] via
  ones/C matmul (no mu^2 cancellation); istd' = Exp(-0.5*Ln(var+eps) +
  ln(g*lw)) with the affine gain folded into the ACT bias host-side.
  PSUM drains for center/var go to GPSIMD; chunked 4x for pipelining.
- current_blue eliminated: Q1 = Wq1@blue + (Wq1/lw0)@Ng0 folded into one
  PSUM accumulation group (Ng0 = lw0*g0*N0 is the layer-0 output tensor).
- out = blue + acc + sum(lw_i*b_i) finished on HOST in f32.
"""

import os
import sys

import numpy as np

if "/opt/trn_rl_repo" not in sys.path:
    sys.path.insert(0, "/opt/trn_rl_repo")

import ml_dtypes

TS = 4
C = 128
NUM_LAYERS = 2
SCALE = float((TS * TS) ** -0.5)
LN_EPS = 1e-5

B, H, W = 4, 128, 128
ROWS = H // 2
KROWS = ROWS + 2 * TS
PW = W + 2 * TS
NTH = ROWS // TS
NTW = W // TS
NTOK = NTH * NTW
NPIX = ROWS * W

_CACHE = {}


def _restride(ap, dim, step):
    b = ap.copy()
    b.ap[dim] = [step, b.ap[dim][1]]
    return b


def _build(reps=1, masked=False, possign=True, unroll=False):
    import contextlib
    import concourse.bass as bass  # noqa: F401
    import concourse.tile as tile
    from concourse import bacc, mybir

    if not getattr(bacc, "_act_tables_patched", False):
        _orig_tables = bacc.get_activation_tables
        _KEEP = "natural_log_exp_and_others"

        def _patched(arch):
            t = _orig_tables(arch)
            mine = t[_KEEP]
            return {
                name: (fns if name == _KEEP else (fns - mine))
                for name, fns in t.items()
            }

        bacc.get_activation_tables = _patched
        bacc._act_tables_patched = True

    F32 = mybir.dt.float32
    BF16 = mybir.dt.bfloat16
    ACTF = mybir.ActivationFunctionType

    nc = bacc.Bacc("TRN2", target_bir_lowering=False, debug=False,
                   num_devices=8)

    d_blueb = nc.dram_tensor("blueb", [C, NPIX], BF16,
                             kind="ExternalInput").ap()
    d_whiteb = nc.dram_tensor("whiteb", [C, KROWS * W], BF16,
                              kind="ExternalInput").ap()
    d_w = nc.dram_tensor("w", [C, 1280], BF16, kind="ExternalInput").ap()
    d_vecs = nc.dram_tensor("vecs", [C, 12], F32, kind="ExternalInput").ap()
    d_consts = nc.dram_tensor("consts", [C, 2], F32,
                              kind="ExternalInput").ap()
    d_out = nc.dram_tensor("out", [C, NPIX], BF16, kind="ExternalOutput").ap()

    with tile.TileContext(nc) as tc:
        with (
            nc.allow_low_precision("bf16 compute by design"),
            tc.tile_pool(name="pp", bufs=1) as pp,
            tc.tile_pool(name="psp", bufs=1, space="PSUM") as psp,
        ):
            blueb = pp.tile([C, NPIX], BF16)            # 16K
            whiteb = pp.tile([C, KROWS * W], BF16)      # 18K
            wts = pp.tile([C, 1280], BF16)              # 2.5K
            vecs = pp.tile([C, 12], F32)
            consts = pp.tile([C, 2], F32)
            eps_t = pp.tile([C, 1], F32)
            Kbuf = pp.tile([C, KROWS, PW], BF16)        # 19.1K
            Vbuf = pp.tile([C, KROWS, PW], BF16)        # 19.1K
            T1q = pp.tile([C, NPIX], BF16)              # 16K  Q / O / Xc
            Ng0 = pp.tile([C, NPIX], BF16)              # 16K  layer-0 out
            Ph = pp.tile([C, 32, 128], BF16)            # 8K   QK products
            T1h = pp.tile([C, 32, 32, 2], BF16)         # 4K   s-pair sums
            T2h = [pp.tile([C, 32, 32], BF16, name=f"t2h{i}")
                   for i in range(2)]                   # 2x2K s sums
            U1h = pp.tile([C, 8, 2, 32], BF16)          # 1K   r-pair sums
            S = pp.tile([C, 9, NTOK], BF16)             # 9K logits/E/A
            axp = pp.tile([C, 9, NTH, NTW, TS], BF16)   # 36K upsampled A
            dL2 = pp.tile([C, 2, NTOK], BF16)           # 2K
            den = pp.tile([C, NTOK], BF16)              # 1K
            dinv = pp.tile([C, NTOK], BF16)             # 1K
            cb = [pp.tile([C, 2048], BF16, name=f"cb{i}") for i in range(3)]
            varb = pp.tile([C, NPIX], BF16)             # 16K var/istd

            ps_a = psp.tile([C, 1024], F32)
            ps_b = psp.tile([C, 2048], F32)
            ps_v = psp.tile([C, 1024], F32)

            nc.sync.dma_start(blueb[:], d_blueb[:])
            nc.sync.dma_start(whiteb[:], d_whiteb[:])
            nc.sync.dma_start(wts[:], d_w[:])
            nc.sync.dma_start(vecs[:], d_vecs[:])
            nc.sync.dma_start(consts[:], d_consts[:])
            nc.vector.memset(eps_t[:], LN_EPS)
            mtop = consts[:, 0:1]
            mbot = consts[:, 1:2]
            # zero x-margins of K/V once (drains never write them)
            for t in (Kbuf, Vbuf):
                m = _restride(
                    t[:, :, 0:TS].unsqueeze(2).broadcast_to(
                        [C, KROWS, 2, TS]), 2, W + TS)
                nc.gpsimd.memset(m, 0.0)

            # weight slices
            def wmat(i):
                return wts[:, 128 * i:128 * (i + 1)]

            WQ = [wmat(0), wmat(3)]
            WK = [wmat(1), wmat(4)]
            WV = [wmat(2), wmat(5)]
            WQG = wmat(6)
            MC = wmat(7)      # I - J/C
            ONESC = wmat(8)   # 1/C
            IDENT = wmat(9)

            qb = [vecs[:, 0:1], vecs[:, 1:2]]
            kb = [vecs[:, 2:3], vecs[:, 3:4]]
            vb = [vecs[:, 4:5], vecs[:, 5:6]]
            lnb = [vecs[:, 6:7], vecs[:, 7:8]]
            sv = [vecs[:, 8:9], vecs[:, 9:10]]

            def emit_kv_conv(li, which):
                w = WK[li] if which == "k" else WV[li]
                bias = kb[li] if which == "k" else vb[li]
                dst = Kbuf if which == "k" else Vbuf
                npx = KROWS * W
                px0 = 0
                while px0 < npx:
                    px1 = min(px0 + 1024, npx)
                    for k in range(px0, px1, 512):
                        nc.tensor.matmul(
                            ps_a[:, k - px0:k - px0 + 512],
                            w, whiteb[:, k:k + 512], start=True, stop=True)
                    o = dst[:, px0 // W:px1 // W, TS:TS + W]
                    i = ps_a[:, 0:px1 - px0].rearrange(
                        "c (h w) -> c h w", w=W)
                    nc.scalar.activation(o, i, ACTF.Identity, bias=bias)
                    px0 = px1
                if masked:
                    nc.vector.tensor_scalar_mul(
                        dst[:, 0:TS, :], dst[:, 0:TS, :], mtop)
                    nc.vector.tensor_scalar_mul(
                        dst[:, ROWS + TS:KROWS, :],
                        dst[:, ROWS + TS:KROWS, :], mbot)

            def emit_q_conv(li):
                for px0 in range(0, NPIX, 1024):
                    for k in range(px0, px0 + 1024, 512):
                        nc.tensor.matmul(
                            ps_a[:, k - px0:k - px0 + 512],
                            WQ[li], blueb[:, k:k + 512],
                            start=True, stop=(li == 0))
                    if li == 1:
                        for k in range(px0, px0 + 1024, 512):
                            nc.tensor.matmul(
                                ps_a[:, k - px0:k - px0 + 512],
                                WQG, Ng0[:, k:k + 512],
                                start=False, stop=True,
                                skip_group_check=True)
                    nc.scalar.activation(
                        T1q[:, px0:px0 + 1024], ps_a[:],
                        ACTF.Identity, bias=qb[li])

            # prologue: layer-0 convs
            emit_kv_conv(0, "k")
            emit_q_conv(0)
            emit_kv_conv(0, "v")

            if unroll:
                loop = contextlib.nullcontext()
                n_bodies = reps
            else:
                loop = (tc.For_i(0, reps, 1) if reps > 1
                        else contextlib.nullcontext())
                n_bodies = 1
            with loop:
              for _body in range(n_bodies):
                for li in range(NUM_LAYERS):
                    nli = 1 - li
                    emit_next = (li == 0) or reps > 1 or unroll

                    # ---- logits: S[c, 3di+dj, t] = blocksum(Q * shift(K))
                    # pixel-layout products + pairwise tree (2x where legal)
                    Qpix = T1q[:].rearrange("c (y x) -> c y x", x=W)

                    def emit_rtree(di, hh, dj):
                        t2v = T2h[dj % 2][:].rearrange(
                            "c (th r) tw -> c th r tw", r=4)
                        eng = nc.gpsimd if dj == 1 else nc.vector
                        eng.tensor_add(
                            U1h[:], t2v[:, :, 0:2, :], t2v[:, :, 2:4, :])
                        so = S[:, 3 * di + dj,
                               256 * hh:256 * hh + 256].rearrange(
                            "c (th tw) -> c th tw", tw=32)
                        nc.vector.tensor_add(
                            so, U1h[:, :, 0, :], U1h[:, :, 1, :])

                    for di in range(3):
                        for hh in range(2):
                            y0 = 32 * hh
                            for dj in range(3):
                                kv = Kbuf[:, 4 * di + y0:4 * di + y0 + 32,
                                          4 * dj:4 * dj + W]
                                nc.vector.tensor_mul(
                                    Ph[:], Qpix[:, y0:y0 + 32, :], kv)
                                P4 = Ph[:].rearrange(
                                    "c y (tw s) -> c y tw s", s=4)
                                nc.vector.tensor_add(
                                    T1h[:], P4[:, :, :, 0:2], P4[:, :, :, 2:4])
                                t2 = T2h[dj % 2]
                                nc.gpsimd.tensor_add(
                                    t2[:], T1h[:, :, :, 0], T1h[:, :, :, 1])
                                if dj >= 1:
                                    emit_rtree(di, hh, dj - 1)
                            emit_rtree(di, hh, 2)
                        nc.scalar.activation(
                            S[:, 3 * di:3 * di + 3].rearrange(
                                "c n t -> c (n t)"),
                            S[:, 3 * di:3 * di + 3].rearrange(
                                "c n t -> c (n t)"),
                            ACTF.Exp, scale=SCALE)

                    # ---- softmax at token res: A = E / den
                    dL1 = Ph[:].rearrange(
                        "c a b -> c (a b)")[:, 0:2048].rearrange(
                        "c (n t) -> c n t", t=NTOK)
                    nc.vector.tensor_add(dL1, S[:, 0:4], S[:, 4:8])
                    nc.vector.tensor_add(dL2[:], dL1[:, 0:2], dL1[:, 2:4])
                    nc.vector.tensor_add(den[:], dL2[:, 0], dL2[:, 1])
                    nc.vector.tensor_add(den[:], den[:], S[:, 8])
                    nc.vector.reciprocal(dinv[:], den[:])
                    nc.vector.tensor_mul(
                        S[:], S[:],
                        dinv[:].unsqueeze(1).broadcast_to([C, 9, NTOK]))
                    # upsample A -> axp: band 0 via one DVE broadcast
                    # copy (2x_2p), bands 1-3 via ACT strided copies
                    nc.vector.tensor_copy(
                        axp[:, :, 0:4, :, :],
                        S[:, :, 0:128].rearrange(
                            "c n (th tw) -> c n th tw", tw=32).unsqueeze(
                            4).broadcast_to([C, 9, 4, 32, 4]))
                    for st in range(1, 4):
                        for s_ in range(4):
                            nc.scalar.copy(
                                axp[:, :, 4 * st:4 * st + 4, :, s_],
                                S[:, :, 128 * st:128 * st + 128].rearrange(
                                    "c n (th tw) -> c n th tw", tw=32))

                    # next layer's K conv hides under combine DVE stream
                    if emit_next:
                        emit_kv_conv(nli, "k")

                    # ---- combine: O = sum_n upsample(A_n) * shift_n(V)
                    # DVE products per strip; PE center-matmul PSUM acc.
                    # Xc lands in Ng0 (layer 0, LN done in place) / dead
                    # Vbuf space (layer 1) so T1q frees after L1 logits
                    # and next-iter Q0 can overlap this iter's tail.
                    Vfl = Vbuf[:].rearrange("c h w -> c (h w)")[:, 0:NPIX]
                    xc = Ng0[:] if li == 0 else Vfl
                    for st in range(4):
                        for n in range(9):
                            di, dj = n // 3, n % 3
                            b = cb[n % 3]
                            vsl = Vbuf[:, 16 * st + 4 * di:
                                       16 * st + 4 * di + 16,
                                       4 * dj:4 * dj + 128].rearrange(
                                "c (th r) x -> c th r x", r=4)
                            asl = axp[:, n, 4 * st:4 * st + 4, :, :].rearrange(
                                "c th tw s -> c th (tw s)").unsqueeze(
                                2).broadcast_to([C, 4, 4, 128])
                            nc.vector.tensor_mul(
                                b[:].rearrange(
                                    "c (th r x) -> c th r x", r=4, x=128),
                                asl, vsl)
                            for c4 in range(4):
                                nc.tensor.matmul(
                                    ps_b[:, 512 * c4:512 * c4 + 512],
                                    MC, b[:, 512 * c4:512 * c4 + 512],
                                    start=(n == 0), stop=(n == 8),
                                    skip_group_check=(n > 0))
                        # PSUM holds centered Xc (M@O = sum M@P_n).
                        # LN rides per-strip: sq from PSUM on ACT, Xc to
                        # SBUF on DVE, var in its own PSUM bank, Ln fuses
                        # the var drain, Ng-mul in place.
                        s0 = 2048 * st
                        nc.scalar.activation(
                            cb[2][:, 0:1024], ps_b[:, 0:1024], ACTF.Square)
                        nc.scalar.activation(
                            cb[2][:, 1024:2048], ps_b[:, 1024:2048],
                            ACTF.Square)
                        nc.scalar.activation(
                            xc[:, s0:s0 + 2048], ps_b[:], ACTF.Copy)
                        for hf in range(2):
                            h0 = 1024 * hf
                            for c4 in range(2):
                                nc.tensor.matmul(
                                    ps_v[:, 512 * c4:512 * c4 + 512],
                                    ONESC,
                                    cb[2][:, h0 + 512 * c4:h0 + 512 * c4 + 512],
                                    start=True, stop=True)
                            nc.scalar.activation(
                                varb[:, s0 + h0:s0 + h0 + 1024], ps_v[:],
                                ACTF.Ln, bias=eps_t[:])
                            nc.scalar.activation(
                                varb[:, s0 + h0:s0 + h0 + 1024],
                                varb[:, s0 + h0:s0 + h0 + 1024],
                                ACTF.Exp, scale=-0.5, bias=lnb[li])
                            nc.vector.tensor_mul(
                                xc[:, s0 + h0:s0 + h0 + 1024],
                                xc[:, s0 + h0:s0 + h0 + 1024],
                                varb[:, s0 + h0:s0 + h0 + 1024])
                    if li == 0:
                        if emit_next:
                            emit_q_conv(1)
                            emit_kv_conv(1, "v")
                    else:
                        if reps > 1:
                            emit_kv_conv(0, "k")
                            emit_q_conv(0)
                        if not possign:
                            nc.vector.tensor_scalar_mul(
                                Ng0[:], Ng0[:], sv[0])
                            nc.vector.tensor_scalar_mul(
                                Vfl, Vfl, sv[1])
                        nc.vector.tensor_add(Ng0[:], Ng0[:], Vfl)
                        if reps > 1:
                            emit_kv_conv(0, "v")

                nc.sync.dma_start(d_out[:], Ng0[:])

    nc.compile()
    return nc


def _prep_inputs(blue, white, q_w, q_b, k_w, k_b, v_w, v_b, ln_g, ln_b,
                 layer_weights):
    bf16 = ml_dtypes.bfloat16
    f32 = np.float32

    blue = np.asarray(blue, f32)
    whiteP = np.zeros((B, C, H + 2 * TS, W), dtype=f32)
    whiteP[:, :, TS:TS + H, :] = np.asarray(white, f32)

    q_w = np.asarray(q_w, f32)
    q_b = np.asarray(q_b, f32)
    k_w = np.asarray(k_w, f32)
    k_b = np.asarray(k_b, f32)
    v_w = np.asarray(v_w, f32)
    v_b = np.asarray(v_b, f32)
    ln_b = np.asarray(ln_b, f32)
    ln_g = np.asarray(ln_g, f32)
    lwv = np.asarray(layer_weights, f32)

    possign = bool(lwv[0] > 0 and lwv[1] > 0 and np.all(ln_g > 0))

    wpack = np.zeros((C, 1280), dtype=bf16)
    wpack[:, 0:128] = q_w[0].T.astype(bf16)
    wpack[:, 128:256] = k_w[0].T.astype(bf16)
    wpack[:, 256:384] = v_w[0].T.astype(bf16)
    wpack[:, 384:512] = q_w[1].T.astype(bf16)
    wpack[:, 512:640] = k_w[1].T.astype(bf16)
    wpack[:, 640:768] = v_w[1].T.astype(bf16)
    if possign:
        # Ng0 = lw0*g0*N0 ; Q1 needs g0*N0 = Ng0/lw0
        wpack[:, 768:896] = (q_w[1].T / lwv[0]).astype(bf16)
    else:
        # Ng0 = |g0|*N0 ; Q1 needs g0*N0 = sign(g0)*Ng0
        wpack[:, 768:896] = (q_w[1].T
                             * np.sign(ln_g[0])[:, None]).astype(bf16)
    wpack[:, 896:1024] = (np.eye(C, dtype=f32) - 1.0 / C).astype(bf16)
    wpack[:, 1024:1152] = np.full((C, C), 1.0 / C, dtype=f32).astype(bf16)
    wpack[:, 1152:1280] = np.eye(C, dtype=f32).astype(bf16)

    vecs = np.zeros((C, 12), dtype=f32)
    vecs[:, 0] = q_b[0]
    vecs[:, 1] = q_b[1] + q_w[1] @ ln_b[0]
    vecs[:, 2] = k_b[0]
    vecs[:, 3] = k_b[1]
    vecs[:, 4] = v_b[0]
    vecs[:, 5] = v_b[1]
    with np.errstate(divide="ignore"):
        if possign:
            vecs[:, 6] = np.log(lwv[0] * ln_g[0])
            vecs[:, 7] = np.log(lwv[1] * ln_g[1])
        else:
            vecs[:, 6] = np.log(np.abs(ln_g[0]))
            vecs[:, 7] = np.log(np.abs(ln_g[1]))
    vecs[:, 8] = lwv[0] * np.sign(ln_g[0])
    vecs[:, 9] = lwv[1] * np.sign(ln_g[1])

    in_maps = []
    for core in range(8):
        b, half = core // 2, core % 2
        y0 = half * ROWS
        consts = np.zeros((C, 2), f32)
        consts[:, 0] = 0.0 if half == 0 else 1.0
        consts[:, 1] = 0.0 if half == 1 else 1.0
        in_maps.append({
            "blueb": np.ascontiguousarray(
                blue[b, :, y0:y0 + ROWS, :]).reshape(C, NPIX).astype(bf16),
            "whiteb": np.ascontiguousarray(
                whiteP[b, :, y0:y0 + KROWS, :]).reshape(
                C, KROWS * W).astype(bf16),
            "w": wpack,
            "vecs": vecs,
            "consts": consts,
        })
    return in_maps, possign


def kernel(**inputs):
    from concourse.bass_utils import run_bass_kernel_spmd

    reps = int(os.environ.get("KBENCH_REPS", "1"))
    masked = bool(
        np.any(np.asarray(inputs["k_b"])) or np.any(np.asarray(inputs["v_b"])))
    in_maps, possign = _prep_inputs(**inputs)
    key = ("nc", reps, masked, possign)
    if key not in _CACHE:
        _CACHE[key] = _build(reps, masked, possign)
    nc = _CACHE[key]

    res = run_bass_kernel_spmd(nc, in_maps, core_ids=list(range(8)))

    blue = np.asarray(inputs["blue"], np.float32)
    ln_b = np.asarray(inputs["ln_b"], np.float32)
    lwv = np.asarray(inputs["layer_weights"], np.float32)
    c0 = (lwv[0] * ln_b[0] + lwv[1] * ln_b[1]).astype(np.float32)
    out = np.empty((B, C, H, W), np.float32)
    for core in range(8):
        b, half = core // 2, core % 2
        y0 = half * ROWS
        enh = np.asarray(res.results[core]["out"],
                         np.float32).reshape(C, ROWS, W)
        out[b, :, y0:y0 + ROWS, :] = (
            blue[b, :, y0:y0 + ROWS, :] + enh + c0[:, None, None])
    return out
